# revision 2
# baseline (speedup 1.0000x reference)
"""Max-pooling over sequence spans — Trainium2 Bass kernel.

Problem: context [B=8, S=4096, H=1024] f32, spans_begin/spans_len [B, 100] i32.
Output [B, 100, H]: per span, max over rows context[b, begin:begin+max(len,1)].

Strategy: pure data-parallel over B (one batch per NeuronCore, 8 cores).
Per core the pooling is done as 64 accumulation steps: for step l, an
indirect DMA gathers row idx[n, l] of the context for every span n into a
[100, 1024] SBUF slab (one 4KB descriptor per span), and the vector engine
folds it into an accumulator with elementwise max. Indices are precomputed
on the host as idx[n, l] = begin[n] + min(l, eff_len[n]-1): steps beyond a
span's length re-read the last valid row, which is a no-op under max.
"""

import sys
import numpy as np

sys.path.insert(0, "/opt/trn_rl_repo")

B, S, H = 8, 4096, 1024
N_SPANS = 100
MAX_LEN = 64
N_CORES = 8

_cache = {}


def _build_program(n_steps, k_bufs, repeat=1):
    import concourse.bass as bass
    import concourse.bacc as bacc
    import concourse.mybir as mybir
    import concourse.tile as tile

    nc = bacc.Bacc("TRN2", target_bir_lowering=False, debug=False,
                   num_devices=N_CORES)
    ctx_d = nc.dram_tensor("ctx", [S, H], mybir.dt.float32, kind="ExternalInput")
    idx_d = nc.dram_tensor("idx", [N_SPANS, n_steps], mybir.dt.int32,
                           kind="ExternalInput")
    out_d = nc.dram_tensor("out", [N_SPANS, H], mybir.dt.float32,
                           kind="ExternalOutput")

    with tile.TileContext(nc) as tc:
        with (
            tc.tile_pool(name="persist", bufs=1) as persist,
            tc.tile_pool(name="slabs", bufs=6) as slabs,
        ):
            idx_t = persist.tile([N_SPANS, n_steps], mybir.dt.int32)
            nc.sync.dma_start(out=idx_t[:], in_=idx_d[:])
            for _ in range(repeat):
                accs = []
                for k in range(k_bufs):
                    acc = persist.tile([N_SPANS, H], mybir.dt.float32,
                                       tag=f"acc{k}")
                    nc.vector.memset(acc[:], -3.0e38)
                    accs.append(acc)
                for l in range(n_steps):
                    slab = slabs.tile([N_SPANS, H], mybir.dt.float32)
                    nc.gpsimd.indirect_dma_start(
                        out=slab[:],
                        out_offset=None,
                        in_=ctx_d[:],
                        in_offset=bass.IndirectOffsetOnAxis(
                            ap=idx_t[:, l:l + 1], axis=0),
                    )
                    acc = accs[l % k_bufs]
                    nc.vector.tensor_tensor(out=acc[:], in0=acc[:], in1=slab[:],
                                            op=mybir.AluOpType.max)
                # fold the k accumulators together
                step = 1
                while step < k_bufs:
                    for k in range(0, k_bufs, 2 * step):
                        if k + step < k_bufs:
                            nc.vector.tensor_tensor(
                                out=accs[k][:], in0=accs[k][:],
                                in1=accs[k + step][:],
                                op=mybir.AluOpType.max)
                    step *= 2
                nc.sync.dma_start(out=out_d[:], in_=accs[0][:])
    nc.compile()
    return nc


def _get_program():
    key = ("v1", MAX_LEN, 2)
    if key not in _cache:
        _cache[key] = _build_program(MAX_LEN, 2)
    return _cache[key]


def kernel(context, spans_begin, spans_len):
    from concourse.bass_utils import run_bass_kernel_spmd

    context = np.ascontiguousarray(context, dtype=np.float32)
    spans_begin = np.asarray(spans_begin, dtype=np.int32)
    spans_len = np.asarray(spans_len, dtype=np.int32)

    eff = np.maximum(spans_len, 1)  # [B, N]
    steps = np.arange(MAX_LEN, dtype=np.int32)  # [L]
    idx = spans_begin[:, :, None] + np.minimum(steps[None, None, :],
                                               eff[:, :, None] - 1)
    idx = np.clip(idx, 0, S - 1).astype(np.int32)  # [B, N, L]

    nc = _get_program()
    in_maps = [{"ctx": context[b], "idx": idx[b]} for b in range(B)]
    res = run_bass_kernel_spmd(nc, in_maps, list(range(N_CORES)))
    out = np.stack([res.results[b]["out"] for b in range(B)], axis=0)
    return out.astype(np.float32)


# revision 7
# speedup vs baseline: 118.8618x; 118.8618x over previous
"""Max-pooling over sequence spans — Trainium2 Bass kernel.

Problem: context [B=8, S=4096, H=1024] f32; spans_begin/spans_len [B, 100] i32.
Output [B, 100, H] f32: out[b, n] = max over rows context[b, begin:begin+max(len,1)].

Sharding: pure data-parallel over the batch axis — one batch row per
NeuronCore, 8 cores, no cross-device communication.

Per-core algorithm (bit-exact f32):
  * Host precomputes gather indices idx[n, l] = begin[n] + min(l, eff_len[n]-1)
    for l in [0, 64).  Steps beyond a span's length re-read its last valid row,
    which is a no-op under max, so no masking is needed on device.
  * Device runs 64 accumulation steps.  Step l issues one indirect DMA
    (gpsimd SWDGE) that gathers row idx[n, l] for every span n into a
    [100, 1024] SBUF slab — one 4 KiB descriptor per span — and the vector
    engine folds the slab into one of two rotating accumulators with
    elementwise f32 max.  Two accumulator chains + 6 slab buffers keep the
    DMA stream and DVE fully overlapped; DVE (fp32 tensor_tensor max runs at
    1 elem/cycle/lane) is the ~70 us bottleneck, with the ~25 MB gather
    stream hidden under it.
  * The two accumulators are folded and written back to DRAM.

kernel() compiles the Bass program on first call (~1 s) and caches it for
the lifetime of the process.  It is value-generic: all span data flows in
as device tensors, so any inputs of the declared shapes work.
"""

import sys
import numpy as np

sys.path.insert(0, "/opt/trn_rl_repo")

B, S, H = 8, 4096, 1024
N_SPANS = 100
MAX_LEN = 64
N_CORES = 8

_cache = {}


def _build_program(n_steps, k_bufs, repeat=1, acc_bf16=False, n_slab_bufs=6,
                   gp_memset=False):
    """Build + compile the per-core SPMD program.

    repeat: replicate the kernel body (used only by timing harnesses;
    repeat-delta isolates per-iteration HW time from call overhead).
    acc_bf16: accumulate in bf16 (2x DVE rate, ~4e-3 rel err) — not used by
    kernel(); kept for experimentation.
    """
    import concourse.bass as bass
    import concourse.bacc as bacc
    import concourse.mybir as mybir
    import concourse.tile as tile

    acc_dt = mybir.dt.bfloat16 if acc_bf16 else mybir.dt.float32

    nc = bacc.Bacc("TRN2", target_bir_lowering=False, debug=False,
                   num_devices=N_CORES)
    ctx_d = nc.dram_tensor("ctx", [S, H], mybir.dt.float32, kind="ExternalInput")
    idx_d = nc.dram_tensor("idx", [N_SPANS, n_steps], mybir.dt.int32,
                           kind="ExternalInput")
    out_d = nc.dram_tensor("out", [N_SPANS, H], mybir.dt.float32,
                           kind="ExternalOutput")

    with tile.TileContext(nc) as tc:
        with (
            tc.tile_pool(name="persist", bufs=1) as persist,
            tc.tile_pool(name="slabs", bufs=n_slab_bufs) as slabs,
        ):
            idx_t = persist.tile([N_SPANS, n_steps], mybir.dt.int32)
            nc.sync.dma_start(out=idx_t[:], in_=idx_d[:])
            for _ in range(repeat):
                accs = []
                for k in range(k_bufs):
                    acc = persist.tile([N_SPANS, H], acc_dt, tag=f"acc{k}")
                    (nc.gpsimd if gp_memset else nc.vector).memset(acc[:],
                                                                   -3.0e38)
                    accs.append(acc)
                for l in range(n_steps):
                    slab = slabs.tile([N_SPANS, H], acc_dt)
                    nc.gpsimd.indirect_dma_start(
                        out=slab[:],
                        out_offset=None,
                        in_=ctx_d[:],
                        in_offset=bass.IndirectOffsetOnAxis(
                            ap=idx_t[:, l:l + 1], axis=0),
                    )
                    acc = accs[l % k_bufs]
                    nc.vector.tensor_tensor(out=acc[:], in0=acc[:],
                                            in1=slab[:],
                                            op=mybir.AluOpType.max)
                step = 1
                while step < k_bufs:
                    for k in range(0, k_bufs, 2 * step):
                        if k + step < k_bufs:
                            nc.vector.tensor_tensor(
                                out=accs[k][:], in0=accs[k][:],
                                in1=accs[k + step][:],
                                op=mybir.AluOpType.max)
                    step *= 2
                if acc_bf16:
                    accf = persist.tile([N_SPANS, H], mybir.dt.float32,
                                        tag="accf")
                    nc.vector.tensor_copy(out=accf[:], in_=accs[0][:])
                    nc.sync.dma_start(out=out_d[:], in_=accf[:])
                else:
                    nc.sync.dma_start(out=out_d[:], in_=accs[0][:])
    nc.compile()
    return nc


def _get_program():
    key = ("v1", MAX_LEN, 2)
    if key not in _cache:
        _cache[key] = _build_program(MAX_LEN, 2)
    return _cache[key]


def _make_indices(spans_begin, spans_len):
    eff = np.maximum(spans_len, 1)                       # [B, N]
    steps = np.arange(MAX_LEN, dtype=np.int32)           # [L]
    idx = spans_begin[:, :, None] + np.minimum(steps[None, None, :],
                                               eff[:, :, None] - 1)
    return np.clip(idx, 0, S - 1).astype(np.int32)       # [B, N, L]


def kernel(context, spans_begin, spans_len):
    from concourse.bass_utils import run_bass_kernel_spmd

    context = np.ascontiguousarray(context, dtype=np.float32)
    spans_begin = np.asarray(spans_begin, dtype=np.int32)
    spans_len = np.asarray(spans_len, dtype=np.int32)
    assert context.shape == (B, S, H), context.shape
    assert spans_begin.shape == (B, N_SPANS), spans_begin.shape

    idx = _make_indices(spans_begin, spans_len)
    nc = _get_program()
    in_maps = [{"ctx": context[b], "idx": idx[b]} for b in range(B)]
    res = run_bass_kernel_spmd(nc, in_maps, list(range(N_CORES)))
    out = np.stack([res.results[b]["out"] for b in range(B)], axis=0)
    return out.astype(np.float32)


# revision 8
# speedup vs baseline: 139.6378x; 1.1748x over previous
"""Max-pooling over sequence spans — Trainium2 Bass kernel.

Problem: context [B=8, S=4096, H=1024] f32; spans_begin/spans_len [B, 100] i32.
Output [B, 100, H] f32: out[b, n] = max over rows context[b, begin:begin+max(len,1)].

Sharding: pure data-parallel over the batch axis — one batch row per
NeuronCore, 8 cores, no cross-device communication.

Per-core algorithm (bit-exact f32):
  * Host precomputes gather indices idx[n, l] = begin[n] + min(l, eff_len[n]-1)
    for l in [0, 64).  Steps beyond a span's length re-read its last valid row,
    which is a no-op under max, so no masking is needed on device.
  * Device runs 64 accumulation steps.  Step l issues one indirect DMA
    (gpsimd SWDGE) that gathers row idx[n, l] for every span n into a
    [100, 1024] SBUF slab — one 4 KiB descriptor per span — and the vector
    engine folds the slab into one of two rotating accumulators with
    elementwise f32 max.  Two accumulator chains + 6 slab buffers keep the
    DMA stream and DVE fully overlapped; DVE (fp32 tensor_tensor max runs at
    1 elem/cycle/lane) is the ~70 us bottleneck, with the ~25 MB gather
    stream hidden under it.
  * The two accumulators are folded and written back to DRAM.

kernel() compiles the Bass program on first call (~1 s) and caches it for
the lifetime of the process.  It is value-generic: all span data flows in
as device tensors, so any inputs of the declared shapes work.
"""

import sys
import numpy as np

sys.path.insert(0, "/opt/trn_rl_repo")

B, S, H = 8, 4096, 1024
N_SPANS = 100
MAX_LEN = 64
N_CORES = 8

_cache = {}


def _build_program(n_steps, k_bufs, repeat=1, acc_bf16=False, n_slab_bufs=6,
                   gp_memset=False):
    """Build + compile the per-core SPMD program.

    repeat: replicate the kernel body (used only by timing harnesses;
    repeat-delta isolates per-iteration HW time from call overhead).
    acc_bf16: accumulate in bf16 (2x DVE rate, ~4e-3 rel err) — not used by
    kernel(); kept for experimentation.
    """
    import concourse.bass as bass
    import concourse.bacc as bacc
    import concourse.mybir as mybir
    import concourse.tile as tile

    acc_dt = mybir.dt.bfloat16 if acc_bf16 else mybir.dt.float32

    nc = bacc.Bacc("TRN2", target_bir_lowering=False, debug=False,
                   num_devices=N_CORES)
    ctx_d = nc.dram_tensor("ctx", [S, H], mybir.dt.float32, kind="ExternalInput")
    idx_d = nc.dram_tensor("idx", [N_SPANS, n_steps], mybir.dt.int32,
                           kind="ExternalInput")
    out_d = nc.dram_tensor("out", [N_SPANS, H], mybir.dt.float32,
                           kind="ExternalOutput")

    with tile.TileContext(nc) as tc:
        with (
            tc.tile_pool(name="persist", bufs=1) as persist,
            tc.tile_pool(name="slabs", bufs=n_slab_bufs) as slabs,
        ):
            idx_t = persist.tile([N_SPANS, n_steps], mybir.dt.int32)
            nc.sync.dma_start(out=idx_t[:], in_=idx_d[:])
            for _ in range(repeat):
                accs = []
                for k in range(k_bufs):
                    acc = persist.tile([N_SPANS, H], acc_dt, tag=f"acc{k}")
                    (nc.gpsimd if gp_memset else nc.vector).memset(acc[:],
                                                                   -3.0e38)
                    accs.append(acc)
                for l in range(n_steps):
                    slab = slabs.tile([N_SPANS, H], acc_dt)
                    nc.gpsimd.indirect_dma_start(
                        out=slab[:],
                        out_offset=None,
                        in_=ctx_d[:],
                        in_offset=bass.IndirectOffsetOnAxis(
                            ap=idx_t[:, l:l + 1], axis=0),
                    )
                    acc = accs[l % k_bufs]
                    nc.vector.tensor_tensor(out=acc[:], in0=acc[:],
                                            in1=slab[:],
                                            op=mybir.AluOpType.max)
                step = 1
                while step < k_bufs:
                    for k in range(0, k_bufs, 2 * step):
                        if k + step < k_bufs:
                            nc.vector.tensor_tensor(
                                out=accs[k][:], in0=accs[k][:],
                                in1=accs[k + step][:],
                                op=mybir.AluOpType.max)
                    step *= 2
                if acc_bf16:
                    accf = persist.tile([N_SPANS, H], mybir.dt.float32,
                                        tag="accf")
                    nc.vector.tensor_copy(out=accf[:], in_=accs[0][:])
                    nc.sync.dma_start(out=out_d[:], in_=accf[:])
                else:
                    nc.sync.dma_start(out=out_d[:], in_=accs[0][:])
    nc.compile()
    return nc


def _get_program(n_steps=MAX_LEN):
    key = ("v1", n_steps, 2)
    if key not in _cache:
        _cache[key] = _build_program(n_steps, 2)
    return _cache[key]


def _make_indices(spans_begin, spans_len, n_steps=MAX_LEN):
    eff = np.maximum(spans_len, 1)                       # [B, N]
    steps = np.arange(n_steps, dtype=np.int32)           # [L]
    idx = spans_begin[:, :, None] + np.minimum(steps[None, None, :],
                                               eff[:, :, None] - 1)
    return np.clip(idx, 0, S - 1).astype(np.int32)       # [B, N, L]


def kernel(context, spans_begin, spans_len):
    from concourse.bass_utils import run_bass_kernel_spmd

    context = np.ascontiguousarray(context, dtype=np.float32)
    spans_begin = np.asarray(spans_begin, dtype=np.int32)
    spans_len = np.asarray(spans_len, dtype=np.int32)
    assert context.shape == (B, S, H), context.shape
    assert spans_begin.shape == (B, N_SPANS), spans_begin.shape

    # Steps beyond the longest effective span are pure duplicates — drop them.
    n_steps = int(min(MAX_LEN, max(1, np.maximum(spans_len, 1).max())))
    idx = _make_indices(spans_begin, spans_len, n_steps)
    nc = _get_program(n_steps)
    in_maps = [{"ctx": context[b], "idx": idx[b]} for b in range(B)]
    res = run_bass_kernel_spmd(nc, in_maps, list(range(N_CORES)))
    out = np.stack([res.results[b]["out"] for b in range(B)], axis=0)
    return out.astype(np.float32)
